# revision 1
# baseline (speedup 1.0000x reference)
"""CostVolume kernel for Trainium2 (8 NeuronCores, batch-sharded).

out[b,h,w,(di,dj)] = mean_c( prv[b,h,w,c] * nxt_pad[b,h+di,w+dj,c] ),  r=4, d=9.

Device strategy (per core, 2 batches):
  - Host prep: prv scaled by 1/C -> bf16, patch-major [b, c, I, J, 128];
    nxt -> bf16, [b, c, 136, 136] zero-padded.
  - Per 16x8-pixel patch: matmul (M=128 pixels, N=384 = 24x16 nxt window)
    contracting c. Chunk1 (K=128) per patch; chunk2 (K=64) of two adjacent
    patches runs CONCURRENTLY on PE row-groups 0-1 / 2-3 via tile_position.
    The chunk2 operands are duplicated onto SBUF partitions 64-127 by
    SBUF->SBUF DMA (fabric bandwidth, no extra HBM traffic).
  - DVE/ACT alternate the PSUM->SBUF bf16 copy. Output: per (b, I) and each
    pair of 8-partition row groups, one DMA of band[16g:16g+16, :, 32g:32g+160]
    (the only window those partitions need) -> 10.5 MB/core instead of 25 MB.
  - Three DMA rings: gpsimd (SWDGE) carries nxt loads, sync carries prv
    loads, scalar carries band stores - so input prefetch for batch 1 is
    never queued behind batch 0's output.
  - Host gathers out[...] = band_g[b,I,g,jp,J, 16*(jp//8) + 16*di + jp%8 + dj].
"""

import numpy as np
import ml_dtypes

B, H, W, C = 16, 128, 128, 192
R = 4
D = 2 * R + 1  # 9
N_CORES = 8
B_LOC = B // N_CORES  # 2
C0 = 128  # first contraction chunk
C1 = C - C0  # 64
PH, PW = 16, 8  # patch size (h, w); PH*PW = 128 = M
WH, WW = PH + 2 * R, PW + 2 * R  # 24, 16 window
NB = WH * WW  # 384 band columns per patch
GP = 8  # row-group pairs per patch (16 partitions each)
GW2 = 160  # band window width per row-group pair
NI = H // PH  # 8 patch rows
NJ = W // PW  # 16 patch cols
HP = H + 2 * R  # 136 padded
NSL = 4  # h-slices per nxt load

_CACHED = {}


def _build_nc():
    import concourse.mybir as mybir
    from concourse.bacc import Bacc
    from concourse.tile import TileContext

    fp32 = mybir.dt.float32
    bf16 = mybir.dt.bfloat16

    nc = Bacc(
        "TRN2",
        target_bir_lowering=False,
        debug=False,
        num_devices=N_CORES,
    )

    prv_d = nc.dram_tensor(
        "prv_t", [B_LOC, C, NI, NJ, PH * PW], bf16, kind="ExternalInput"
    )
    nxt_d = nc.dram_tensor("nxt_p", [B_LOC, C, HP, HP], bf16, kind="ExternalInput")
    band_d = nc.dram_tensor(
        "band", [B_LOC, NI, PH * PW, NJ, NB], bf16, kind="ExternalOutput"
    )

    slices = [(HP * s // NSL, HP * (s + 1) // NSL) for s in range(NSL)]

    with TileContext(nc) as tc:
        with (
            tc.tile_pool(name="nxt0_pool", bufs=2) as nxt0_pool,
            tc.tile_pool(name="nxt1_pool", bufs=2) as nxt1_pool,
            tc.tile_pool(name="prv_pool", bufs=2) as prv_pool,
            tc.tile_pool(name="band_pool", bufs=2) as band_pool,
            tc.tile_pool(name="psum_pool", bufs=8, space="PSUM") as psum_pool,
        ):
            for b in range(B_LOC):
                n0 = nxt0_pool.tile([C0, HP, HP], bf16, tag="nxt_c0")
                n1 = nxt1_pool.tile([C0, HP, HP], bf16, tag="nxt_c1")
                for lo, hi in slices:
                    nc.gpsimd.dma_start(n0[:, lo:hi, :], nxt_d[b, 0:C0, lo:hi, :])
                    nc.gpsimd.dma_start(
                        n1[0:C1, lo:hi, :], nxt_d[b, C0:C, lo:hi, :]
                    )
                    nc.gpsimd.dma_start(n1[C1:C0, lo:hi, :], n1[0:C1, lo:hi, :])

                prv_tiles = {}

                def load_prv(i):
                    p0 = prv_pool.tile([C0, NJ, PH * PW], bf16, tag="prv_c0")
                    p1 = prv_pool.tile([C0, NJ, PH * PW], bf16, tag="prv_c1")
                    nc.sync.dma_start(p0[:], prv_d[b, 0:C0, i])
                    nc.sync.dma_start(p1[0:C1], prv_d[b, C0:C, i])
                    nc.sync.dma_start(p1[C1:C0], p1[0:C1])
                    prv_tiles[i] = (p0, p1)

                load_prv(0)
                load_prv(1)

                for i in range(NI):
                    if i + 2 < NI:
                        load_prv(i + 2)
                    p0, p1 = prv_tiles.pop(i)
                    band = band_pool.tile([PH * PW, NJ, NB], bf16, tag="band_sb")
                    r0 = slice(i * PH, i * PH + WH)
                    for t in range(NJ // 2):
                        ja, jb = 2 * t, 2 * t + 1
                        ca = slice(ja * PW, ja * PW + WW)
                        cb = slice(jb * PW, jb * PW + WW)
                        psa = psum_pool.tile([PH * PW, NB], fp32, tag="band_ps")
                        psb = psum_pool.tile([PH * PW, NB], fp32, tag="band_ps")
                        nc.tensor.matmul(
                            psa[:], p0[:, ja, :], n0[:, r0, ca],
                            start=True, stop=False,
                        )
                        nc.tensor.matmul(
                            psb[:], p0[:, jb, :], n0[:, r0, cb],
                            start=True, stop=False,
                        )
                        nc.tensor.matmul(
                            psa[:], p1[0:C1, ja, :], n1[0:C1, r0, ca],
                            start=False, stop=True, tile_position=(0, 0),
                        )
                        nc.tensor.matmul(
                            psb[:], p1[C1:C0, jb, :], n1[C1:C0, r0, cb],
                            start=False, stop=True, tile_position=(64, 0),
                        )
                        nc.vector.tensor_copy(band[:, ja, :], psa[:])
                        nc.scalar.copy(band[:, jb, :], psb[:])
                    nc.scalar.dma_start(band_d[b, i], band[:])

    nc.finalize()
    return nc


def _get_nc():
    if "nc" not in _CACHED:
        _CACHED["nc"] = _build_nc()
    return _CACHED["nc"]


def _host_prep(prv, nxt):
    """prv: scale by 1/C, bf16, patch-major [b, c, I, J, 128].
    nxt: bf16 [b, c, 136, 136] zero-padded."""
    bf16 = ml_dtypes.bfloat16
    prv_t = (np.asarray(prv, dtype=np.float32) * (1.0 / C)).transpose(0, 3, 1, 2)
    prv_t = prv_t.reshape(B, C, NI, PH, NJ, PW).transpose(0, 1, 2, 4, 3, 5)
    prv_t = np.ascontiguousarray(prv_t.reshape(B, C, NI, NJ, PH * PW)).astype(bf16)
    nxt_t = np.asarray(nxt, dtype=np.float32).transpose(0, 3, 1, 2).astype(bf16)
    nxt_p = np.zeros((B, C, HP, HP), dtype=bf16)
    nxt_p[:, :, R:R + H, R:R + W] = nxt_t
    return prv_t, nxt_p


def _make_in_maps(prv, nxt):
    prv_t, nxt_p = _host_prep(prv, nxt)
    return [
        {
            "prv_t": prv_t[i * B_LOC:(i + 1) * B_LOC],
            "nxt_p": nxt_p[i * B_LOC:(i + 1) * B_LOC],
        }
        for i in range(N_CORES)
    ]


# gather index: n[p=(i,j), di, dj] = (i+di)*WW + (j+dj)
_ii, _jj = np.meshgrid(np.arange(PH), np.arange(PW), indexing="ij")
_di, _dj = np.meshgrid(np.arange(D), np.arange(D), indexing="ij")
_GIDX = (
    (_ii.reshape(-1)[:, None, None] + _di[None]) * WW
    + (_jj.reshape(-1)[:, None, None] + _dj[None])
).reshape(1, 1, 1, PH * PW, D * D)  # [1,1,1,128,81]


def _gather_band(band):
    """band: [B_LOC, NI, 128, NJ, NB] bf16 -> out [B_LOC, H, W, D*D] f32."""
    band = np.asarray(band, dtype=np.float32)
    band = band.transpose(0, 1, 3, 2, 4)  # [b, I, J, p, NB]
    idx = np.broadcast_to(_GIDX, band.shape[:3] + (PH * PW, D * D))
    out = np.take_along_axis(band, idx, axis=-1)  # [b, I, J, 128, 81]
    out = out.reshape(B_LOC, NI, NJ, PH, PW, D * D)
    out = out.transpose(0, 1, 3, 2, 4, 5)  # [b, I, i, J, j, 81]
    return np.ascontiguousarray(out.reshape(B_LOC, H, W, D * D))


def kernel(prv, nxt, search_range):
    from concourse.bass_utils import run_bass_kernel_spmd

    assert int(search_range) == R
    prv = np.asarray(prv)
    nxt = np.asarray(nxt)
    assert prv.shape == (B, H, W, C), prv.shape

    in_maps = _make_in_maps(prv, nxt)

    nc = _get_nc()
    res = run_bass_kernel_spmd(nc, in_maps, list(range(N_CORES)))

    out = np.empty((B, H, W, D * D), dtype=np.float32)
    for i in range(N_CORES):
        out[i * B_LOC:(i + 1) * B_LOC] = _gather_band(res.results[i]["band"])
    return out

